# revision 53
# baseline (speedup 1.0000x reference)
"""Trainium2 Bass kernel for nn_Attention_54150947668207 (sparse channel attention).

Algorithm restructure (verified exact vs reference in fp32, rel 3.4e-7):
  - dwconv3x3 per channel on q,k,v (depthwise, SAME pad)
  - per (batch,head): attn = normalize(q) @ normalize(k)^T over pixels; the 4
    top-k masked softmaxes combine into ONE matrix A_comb = sum_i w_i*softmax_i
    (top-k via rank-count, col-scaled before ranking; exp needs no max-sub)
  - M_combT = blockdiag(A_comb)^T @ Wproj^T folds all four attn@v matmuls AND
    the 1x1 projection into ONE [384,384]@[384,px] matmul per pixel shard.

Sharding (8 cores, SPMD):
  - attention phase: core i handles batch i//4, heads {2*(i%4), 2*(i%4)+1}
  - projection phase: core i handles batch i//4, image rows 32*(i%4)..+32
  - connected by one AllGather of M_combT ([96,384] f16) in 4-core groups.

v2 schedule (from trace analysis of v1 at 280us):
  - PE stream kept dense & in one program order: qk dwconv chunks -> attn
    matmuls -> v-group0 (all-9-taps PE; covers the post-attn DVE/ACT wait) ->
    M matmuls -> v-groups 1,2 -> warm-fill -> final matmuls. Keeps HAM at
    full clock (v1 spent 102us at half clock) and fills the collective window.
  - norm reciprocals + rk broadcast DRAM bounce issued during the attn matmul
    phase (their latency fully hidden).
  - A1 head-1 block extracted straight from PSUM with a base-partition-32
    ACT copy (overwritten rows fixed by the head-0 copy) - no DRAM bounce.
  - diag-weight tables host-pre-transposed -> contiguous DMA loads.
  - input loads prefetched: first chunks on HWDGE (sync), rest on SWDGE
    (gpsimd); all dwconv transposes ride the sync HWDGE ring.
"""
import sys

for _p in ("/opt/trn_rl_repo",):
    if _p not in sys.path:
        sys.path.insert(0, _p)

import numpy as np
from contextlib import ExitStack

import concourse.bass as bass
import concourse.tile as tile
from concourse import mybir
from concourse.bass_utils import run_bass_kernel_spmd

F32 = mybir.dt.float32
F16 = mybir.dt.float16
AOT = mybir.AluOpType
ACTF = mybir.ActivationFunctionType

C = 384
HEADS = 8
CH = 48          # channels per head
H = W = 128
HW = H * W
B = 2
C2 = 96          # channels per core in attention phase (2 heads)
KS = (CH // 2, CH * 2 // 3, CH * 3 // 4, CH * 4 // 5)  # 24, 32, 36, 38

# tap order t = 3*ky + kx, offsets (dy,dx) = (ky-1, kx-1)
N_TAPS = 9
CFG = {
    "pe_taps_qk": (0, 1, 3, 4, 5, 7),   # taps done on PE (fp32 psum acc)
    "pe_taps_v": (0, 1, 3, 4, 5, 7),
    "split_waits": True,
    "max_waits": 1,
    "warm_a": 65,        # junk matmuls filling the post-attn PE window
    "warm_b": 75,        # junk matmuls bridging the collective window
    "pads_bufs": 3,
}

NCHUNK = 4            # q/k processed in 4 chunks of 32 rows
ROWS_PER_CHUNK = 32
CHUNK_PX = ROWS_PER_CHUNK * W   # 4096
SUB = 512             # psum sub-chunk width for PE dwconv
VROWS = 32            # v shard rows per core
VPX = VROWS * W       # 4096


def _split_multi_waits(nc, max_waits=1):
    """walrus in this container accepts limited sync waits per instruction;
    split extras into preceding single-wait NoOps on the same engine."""
    n = 0
    for f in nc.m.functions:
        for blk in f.blocks:
            new_insts = []
            for inst in blk.instructions:
                si = getattr(inst, "sync_info", None)
                # TENSOR_TENSOR_REDUCE's long encoding leaves no room for any
                # sync wait ("ISA wrong length" at codegen) — strip them all.
                mw = 0 if isinstance(inst, mybir.InstTensorTensorReduce) else max_waits
                if si is not None and si.on_wait and len(si.on_wait) > mw:
                    waits = list(si.on_wait)
                    if mw == 0:
                        wlist, si.on_wait = waits, []
                    else:
                        wlist, si.on_wait = waits[:-mw], waits[-mw:]
                    for wcond in wlist:
                        nop = mybir.InstNoOp(
                            name=f"I-waitsplit-{nc.next_id()}",
                            ins=[], outs=[],
                            engine=inst.engine,
                            sync_info=mybir.SyncInfo(on_wait=[wcond], on_update=[]),
                        )
                        new_insts.append(nop)
                        n += 1
                new_insts.append(inst)
            blk.instructions = new_insts
    return n


def _emit_dwconv(nc, psum_dw, xpad, w9, diags, dwp, out_tag,
                 npart, pe_taps, nrows, out_final=None, tmp_pool=None,
                 mid_emit=None, half_done=None, split_halves=True):
    """Depthwise 3x3 over nrows output rows.

    PE taps accumulate in fp32 PSUM (diag matmuls); remaining taps run as
    DVE tensor_scalar(4x) + tensor_tensor(2x) chains.
    Returns the final output AP ([npart, nrows*W] f16).
    """
    dve_taps = [t for t in range(N_TAPS) if t not in pe_taps]
    npx = nrows * W
    nsub = npx // 512
    rows_per_sub = 512 // W  # 4

    def shifted(t, r_lo, nr):
        ky, kx = divmod(t, 3)
        return xpad[:, r_lo + ky:r_lo + ky + nr, kx:kx + W]

    _ctr = [0]

    def alloc(tag, pool=None, rows=None):
        _ctr[0] += 1
        t = (pool or dwp).tile([npart, (rows or nrows) * W], F16, tag=tag,
                               name=f"{out_tag}_{tag}{_ctr[0]}")
        return t[:]

    n_dve = len(dve_taps)
    assert pe_taps
    halved_pe_out = split_halves and n_dve > 0
    if halved_pe_out:
        # two half-row tiles as PE-drain targets (halves the tag-A footprint)
        hr0 = nrows // 2
        cur_h = [alloc(out_tag + "A", rows=hr0), alloc(out_tag + "A", rows=hr0)]
        oc3_h = [c.rearrange("p (r w) -> p r w", w=W) for c in cur_h]
    else:
        cur = (out_final if (n_dve == 0 and out_final is not None)
               else alloc(out_tag + ("F" if n_dve == 0 else "A")))
        oc3 = cur.rearrange("p (r w) -> p r w", w=W)
    for s in range(nsub):
        r_lo = s * rows_per_sub
        ps = psum_dw.tile([npart, 512], F32, tag="psdw")
        for i, t in enumerate(pe_taps):
            nc.tensor.matmul(
                ps[:], diags[t], shifted(t, r_lo, 4),
                start=(i == 0), stop=(i == len(pe_taps) - 1))
        if halved_pe_out:
            half = 0 if s < nsub // 2 else 1
            rr = r_lo - half * (nrows // 2)
            nc.scalar.copy(oc3_h[half][:, rr:rr + rows_per_sub, :], ps[:])
        else:
            nc.scalar.copy(oc3[:, r_lo:r_lo + rows_per_sub, :], ps[:])
        if s == nsub // 2 - 1 and mid_emit is not None:
            mid_emit()
    if n_dve == 0:
        return cur
    if not split_halves:
        # single whole-rows chain (v groups: no squares needed, and the
        # bufs=1 v pool cannot keep the PE-out tile alive across two halves)
        oc3 = cur.rearrange("p (r w) -> p r w", w=W)
        flip = 0
        for j, t in enumerate(dve_taps):
            last = (j == n_dve - 1)
            nxt = (out_final if (last and out_final is not None)
                   else alloc(out_tag + "BA"[flip]))
            flip ^= 1
            no3 = nxt.rearrange("p (r w) -> p r w", w=W)
            tmp = alloc("dwtmp", pool=tmp_pool)
            tm3 = tmp.rearrange("p (r w) -> p r w", w=W)
            nc.vector.tensor_scalar(
                tm3, shifted(t, 0, nrows), w9[:, t:t + 1], None, AOT.mult)
            nc.vector.tensor_tensor(no3, tm3, oc3, AOT.add)
            cur, oc3 = nxt, no3
        return cur
    # DVE tap chain split into row-halves so each half's square can slot into
    # the ACT queue as soon as that half's chain is done.
    hr = nrows // 2
    for half in range(2):
        r0 = half * hr
        sub = cur_h[half]
        sc3 = sub.rearrange("p (r w) -> p r w", w=W)
        flip = 0
        for j, t in enumerate(dve_taps):
            last = (j == n_dve - 1)
            if last and out_final is not None:
                nxt = out_final[:, r0 * W:(r0 + hr) * W]
            elif last:
                if half == 0:
                    fin_full = alloc(out_tag + "F", rows=nrows)
                nxt = fin_full[:, r0 * W:(r0 + hr) * W]
            else:
                nxt = alloc(out_tag + "BA"[flip], rows=hr)
            flip ^= 1
            no3 = nxt.rearrange("p (r w) -> p r w", w=W)
            tmp = alloc("dwtmp", pool=tmp_pool, rows=hr)
            tm3 = tmp.rearrange("p (r w) -> p r w", w=W)
            nc.vector.tensor_scalar(
                tm3, shifted(t, r0, hr), w9[:, t:t + 1], None, AOT.mult)
            nc.vector.tensor_tensor(no3, tm3, sc3, AOT.add)
            sub, sc3 = nxt, no3
        if half_done is not None:
            half_done(half, (fin_full if out_final is None else out_final))
    return fin_full if out_final is None else out_final


def build_kernel():
    nc = bass.Bass("TRN2", target_bir_lowering=False, debug=False, num_devices=8)

    # ---- DRAM I/O ----
    qs = nc.declare_dram_parameter("qs", [C2, 130, 130], F16, isOutput=False)
    ks = nc.declare_dram_parameter("ks", [C2, 130, 130], F16, isOutput=False)
    vs = nc.declare_dram_parameter("vs", [C, 34, 130], F16, isOutput=False)
    wq9 = nc.declare_dram_parameter("wq9", [C2, 9], F32, isOutput=False)
    wk9 = nc.declare_dram_parameter("wk9", [C2, 9], F32, isOutput=False)
    wv9 = nc.declare_dram_parameter("wv9", [128, 3, 9], F32, isOutput=False)
    # host pre-transposed: [c, a, t, e] and [c, t, ct, e] (all 9 v taps)
    dgqk = nc.declare_dram_parameter("dgqk", [C2, 2, 9, C2], F16, isOutput=False)
    dgv = nc.declare_dram_parameter("dgv", [128, 9, 3, 128], F16, isOutput=False)
    wpT0 = nc.declare_dram_parameter("wpT0", [CH, C], F16, isOutput=False)
    wpT1 = nc.declare_dram_parameter("wpT1", [CH, C], F16, isOutput=False)
    tempv = nc.declare_dram_parameter("tempv", [C2, 1], F32, isOutput=False)
    attwv = nc.declare_dram_parameter("attwv", [C2, 4], F32, isOutput=False)
    ksv = nc.declare_dram_parameter("ksv", [C2, 4], F32, isOutput=False)
    out_ext = nc.declare_dram_parameter("out", [3, 128, VPX], F16, isOutput=True)

    with tile.TileContext(nc) as tc, ExitStack() as ctx:
        pool = ctx.enter_context(tc.tile_pool(name="sbuf", bufs=1))
        pads = ctx.enter_context(tc.tile_pool(name="pads", bufs=CFG["pads_bufs"]))
        vpads = ctx.enter_context(tc.tile_pool(name="vpads", bufs=2))
        dwp = ctx.enter_context(tc.tile_pool(name="dwp", bufs=2))
        vwp = ctx.enter_context(tc.tile_pool(name="vwp", bufs=1))
        psum_dw = ctx.enter_context(tc.tile_pool(name="psdw", bufs=4, space="PSUM"))
        psum_a = ctx.enter_context(tc.tile_pool(name="psa", bufs=1, space="PSUM"))
        psum_o = ctx.enter_context(tc.tile_pool(name="pso", bufs=2, space="PSUM"))
        obuf = ctx.enter_context(tc.tile_pool(name="obuf", bufs=3))
        dram = ctx.enter_context(tc.tile_pool(name="dram", bufs=1, space="DRAM"))

        # ---- all input loads ride the gpsimd SWDGE ring; the sync HWDGE ring
        # carries ONLY transposes + small bounces + output stores, so a
        # WAR-delayed chunk load can never block a transpose (which blocks
        # eagerly-scheduled attn matmuls on PE).
        # first chunk + diag tables on HWDGE(sync) for the ~0.4us faster
        # first-byte: nothing else occupies the sync ring until transpose 0
        xqk = {}
        t = pads.tile([C2, 34, 130], F16, tag="pad", name="xq0")
        nc.sync.dma_start(t[:], qs.ap()[:, 0:34, :])
        xqk[("q", 0)] = t
        dgqk_t = pool.tile([C2, 2, 9, C2], F16, tag="dgqk")
        nc.sync.dma_start(dgqk_t[:], dgqk.ap())
        w9q = pool.tile([C2, 9], F32); nc.sync.dma_start(w9q[:], wq9.ap())
        w9k = pool.tile([C2, 9], F32); nc.sync.dma_start(w9k[:], wk9.ap())
        t = pads.tile([C2, 34, 130], F16, tag="pad", name="xk0")
        nc.sync.dma_start(t[:], ks.ap()[:, 0:34, :])
        xqk[("k", 0)] = t
        for nm, src in (("q", qs), ("k", ks)):
            t = pads.tile([C2, 34, 130], F16, tag="pad", name=f"x{nm}1")
            nc.gpsimd.dma_start(t[:], src.ap()[:, ROWS_PER_CHUNK:ROWS_PER_CHUNK + 34, :])
            xqk[(nm, 1)] = t

        vpad = []
        for ct in range(2):
            vp = vpads.tile([128, 34, 130], F16, tag="vpad")
            nc.gpsimd.dma_start(vp[:], vs.ap()[128 * ct:128 * (ct + 1), :, :])
            vpad.append(vp)
        dgv_t = pool.tile([128, 9, 3, 128], F16, tag="dgvt")
        nc.gpsimd.dma_start(dgv_t[:], dgv.ap())
        w9v = pool.tile([128, 3, 9], F32); nc.gpsimd.dma_start(w9v[:], wv9.ap())
        wp0 = pool.tile([CH, C], F16); nc.gpsimd.dma_start(wp0[:], wpT0.ap())
        wp1 = pool.tile([CH, C], F16); nc.gpsimd.dma_start(wp1[:], wpT1.ap())
        tmpv = pool.tile([C2, 1], F32); nc.gpsimd.dma_start(tmpv[:], tempv.ap())
        attw = pool.tile([C2, 4], F32); nc.gpsimd.dma_start(attw[:], attwv.ap())
        ks_t = pool.tile([C2, 4], F32); nc.gpsimd.dma_start(ks_t[:], ksv.ap())
        for ci in range(2, NCHUNK):
            r0 = ci * ROWS_PER_CHUNK
            for nm, src in (("q", qs), ("k", ks)):
                t = pads.tile([C2, 34, 130], F16, tag="pad", name=f"x{nm}{ci}")
                nc.gpsimd.dma_start(t[:], src.ap()[:, r0:r0 + 34, :])
                xqk[(nm, ci)] = t
        # v group 2 rides the pads rotation (reuses xq3's slot -> loads once
        # chunk-3 q dwconv is done; needed much later, at v-group-2 time)
        vp2 = pads.tile([128, 34, 130], F16, tag="pad", name="vp2")
        nc.gpsimd.dma_start(vp2[:], vs.ap()[256:384, :, :])
        vpad.append(vp2)

        diag_q = {t: dgqk_t[:, 0, t, :] for t in range(9)}
        diag_k = {t: dgqk_t[:, 1, t, :] for t in range(9)}
        diag_v = {(t, ct): dgv_t[:, t, ct, :]
                  for t in range(9) for ct in range(3)}

        # ---- q/k dwconv, 4 chunks; transposes on sync HWDGE ----
        # Squares (for the L2 norms) run on ACT per row-half. Half-0's square
        # is emitted right after its half-chain (runs while half-1 chains on
        # DVE); half-1's square is DEFERRED into the middle of the next
        # tensor's PSUM-drain stream so it never blocks those drains (the
        # v4/v5 chunk-boundary stall).
        sumsq = {"q": [], "k": []}
        vdw = pool.tile([128, 3, VPX], F16, tag="vdw")
        qT = pool.tile([128, 128, C2], F16, tag="qT")
        kT = pool.tile([128, 128, C2], F16, tag="kT")
        pending_sq = [None]
        HRW = (ROWS_PER_CHUNK // 2) * W

        def flush_pending():
            if pending_sq[0] is not None:
                emit = pending_sq[0]
                pending_sq[0] = None
                emit()

        for ci in range(NCHUNK):
            dws = {}
            for name in ("q", "k"):
                w9_ = w9q if name == "q" else w9k
                dg_ = diag_q if name == "q" else diag_k
                taps = CFG["pe_taps_qk"] if ci < NCHUNK - 1 else tuple(range(9))

                def make_sq(name, ci, half, fin):
                    sl = fin[:, half * HRW:(half + 1) * HRW]
                    ss = pool.tile([C2, 1], F32, tag=f"ss_{name}{ci}h{half}")
                    sumsq[name].append(ss)
                    sq = dwp.tile([C2, HRW], F16, tag="dwtmp",
                                  name=f"sq_{name}{ci}h{half}")

                    def emit():
                        nc.scalar.activation(sq[:], sl, ACTF.Square,
                                             accum_out=ss[:])
                    return emit

                def half_done(half, fin, name=name, ci=ci):
                    emit = make_sq(name, ci, half, fin)
                    if half == 0:
                        emit()
                    else:
                        pending_sq[0] = emit

                dw = _emit_dwconv(nc, psum_dw, xqk[(name, ci)], w9_, dg_,
                                  dwp, "dw", C2, taps, ROWS_PER_CHUNK,
                                  mid_emit=flush_pending, half_done=half_done)
                dws[name] = dw
                if len(taps) == 9:  # all-PE chunk: squares inline after drains
                    flush_pending()
                    for half in range(2):
                        make_sq(name, ci, half, dw)()
            nc.sync.dma_start_transpose(qT[:, 32 * ci:32 * ci + 32, :], dws["q"])
            nc.sync.dma_start_transpose(kT[:, 32 * ci:32 * ci + 32, :], dws["k"])
        flush_pending()

        # ---- norms: total sumsq -> rq, rk, rsc, Bc (latency hides under attn)
        nq2 = pool.tile([C2, 1], F32, tag="nq2")
        nk2 = pool.tile([C2, 1], F32, tag="nk2")
        for name, tgt in (("q", nq2), ("k", nk2)):
            parts = sumsq[name]
            nc.vector.tensor_tensor(tgt[:], parts[0][:], parts[1][:], AOT.add)
            for p in parts[2:]:
                nc.vector.tensor_tensor(tgt[:], tgt[:], p[:], AOT.add)
        rq = pool.tile([C2, 1], F32, tag="rq")
        rk = pool.tile([C2, 1], F32, tag="rk")
        for src2, dst in ((nq2, rq), (nk2, rk)):
            nc.scalar.sqrt(dst[:], src2[:])
            nc.vector.reciprocal(dst[:], dst[:])
        rk_dram = dram.tile([C2, 1], F32)
        nc.sync.dma_start(rk_dram[:], rk[:])
        Bc = pool.tile([C2, CH], F32, tag="Bc")
        rkd = rk_dram[:].rearrange("p one -> (p one)")
        nc.sync.dma_start(
            Bc[0:CH, :],
            rkd[0:CH].rearrange("(x e) -> x e", x=1).broadcast_to([CH, CH]))
        nc.sync.dma_start(
            Bc[CH:C2, :],
            rkd[CH:C2].rearrange("(x e) -> x e", x=1).broadcast_to([CH, CH]))
        rsc = pool.tile([C2, 1], F32, tag="rsc")
        nc.vector.tensor_tensor(rsc[:], rq[:], tmpv[:], AOT.mult)

        # ---- attention matmuls: virtual-time 1.0ms pushes them AFTER the
        # whole qk dwconv phase in the PE queue — otherwise the scheduler
        # parks them at chunk boundaries where they stall on the transpose
        # that trails each chunk's DVE tap chain.
        ps_attn = psum_a.tile([C2, C2], F32, tag="psattn")
        with tc.tile_wait_until(1.0):
            for j in range(128):
                nc.tensor.matmul(ps_attn[:], qT[:, j, :], kT[:, j, :],
                                 start=(j == 0), stop=(j == 127))

        # ---- post-attention (small, DVE/ACT) ----
        # A1[r, d] = attn[r, head(r)*48 + d]; head1 block via base-partition-32
        # psum copy (rows 32:48 garbage, then overwritten by the head0 copy)
        A1 = pool.tile([C2, CH], F32, tag="A1")
        nc.scalar.copy(A1[32:64, :], ps_attn[32:64, CH:C2])
        nc.scalar.copy(A1[64:C2, :], ps_attn[64:C2, CH:C2])
        nc.scalar.copy(A1[0:CH, :], ps_attn[0:CH, 0:CH])
        nc.vector.tensor_tensor(A1[:], A1[:], Bc[:], AOT.mult)
        # E before G: ACT exponentiates while DVE does the rank count
        E = pool.tile([C2, CH], F32, tag="E")
        nc.scalar.activation(E[:], A1[:], ACTF.Exp, scale=rsc[:])
        # rank count: G[r, d, e] = A1[r, e] > A1[r, d]  (free dims d,e)
        G = pool.tile([C2, CH, CH], F16, tag="G")
        in_e = A1[:].rearrange("p (x e) -> p x e", x=1).broadcast_to([C2, CH, CH])
        in_d = A1[:].rearrange("p (d x) -> p d x", x=1).broadcast_to([C2, CH, CH])
        nc.vector.tensor_tensor(G[:], in_e, in_d, AOT.is_gt)
        cnt = pool.tile([C2, CH], F32, tag="cnt")
        nc.vector.tensor_reduce(cnt[:], G[:], axis=mybir.AxisListType.X, op=AOT.add)
        # 4 masked softmaxes, batched over the k axis ([C2, 4, CH] tiles)
        M4 = pool.tile([C2, 4, CH], F16, tag="M4")
        cnt_b = cnt[:].rearrange("p (x e) -> p x e", x=1).broadcast_to([C2, 4, CH])
        ks_b = ks_t[:].rearrange("p (d x) -> p d x", x=1).broadcast_to([C2, 4, CH])
        nc.vector.tensor_tensor(M4[:], cnt_b, ks_b, AOT.is_lt)
        N4 = pool.tile([C2, 4, CH], F32, tag="N4")
        E_b = E[:].rearrange("p (x e) -> p x e", x=1).broadcast_to([C2, 4, CH])
        nc.vector.tensor_tensor(N4[:], E_b, M4[:], AOT.mult)
        den4 = pool.tile([C2, 4], F32, tag="den4")
        nc.vector.tensor_reduce(den4[:], N4[:], axis=mybir.AxisListType.X,
                                op=AOT.add)
        rw4 = pool.tile([C2, 4], F32, tag="rw4")
        nc.vector.reciprocal(rw4[:], den4[:])
        nc.vector.tensor_tensor(rw4[:], rw4[:], attw[:], AOT.mult)
        P4 = pool.tile([C2, 4, CH], F32, tag="P4")
        rw_b = rw4[:].rearrange("p (d x) -> p d x", x=1).broadcast_to([C2, 4, CH])
        nc.vector.tensor_tensor(P4[:], N4[:], rw_b, AOT.mult)
        t01 = pool.tile([C2, CH], F32, tag="t01")
        nc.vector.tensor_tensor(t01[:], P4[:, 0, :], P4[:, 1, :], AOT.add)
        t23 = pool.tile([C2, CH], F32, tag="t23")
        nc.vector.tensor_tensor(t23[:], P4[:, 2, :], P4[:, 3, :], AOT.add)
        Acc = pool.tile([C2, CH], F16, tag="Acc")
        nc.vector.tensor_tensor(Acc[:], t01[:], t23[:], AOT.add)
        # head1 rows to base partition 0 (head0 is Acc[0:CH] in place)
        Ah1 = pool.tile([CH, CH], F16, tag="Ah1")
        nc.gpsimd.dma_start(Ah1[:], Acc[CH:C2, :])

        # ---- v dwconv: virtual 0.9ms = right after the qk phase, before the
        # attn block — the scheduler uses it to cover the last chunk's
        # DVE-chain + transpose tail.
        with tc.tile_wait_until(0.9):
            _emit_dwconv(nc, psum_dw, vpad[0], w9v[:, 0, :],
                         {t: diag_v[(t, 0)] for t in range(9)}, vwp, "vw",
                         128, tuple(range(9)), VROWS, out_final=vdw[:, 0, :])

        # ---- M_combT partials -> DRAM -> AllGather within 4-core groups
        b_in = dram.tile([C2, C], F16)
        b_out = dram.tile([4, C2, C], F16)
        for h, (ah, wp) in enumerate(((Acc[0:CH, :], wp0), (Ah1[:], wp1))):
            ps = psum_a.tile([CH, C], F32, tag="psmc")
            nc.tensor.matmul(ps[:], ah, wp[:], start=True, stop=True)
            mt_h = pool.tile([CH, C], F16, tag=f"mth{h}")
            nc.scalar.copy(mt_h[:], ps[:])
            nc.sync.dma_start(b_in[CH * h:CH * (h + 1), :], mt_h[:])
        nc.gpsimd.collective_compute(
            "AllGather", AOT.bypass,
            replica_groups=[[0, 1, 2, 3], [4, 5, 6, 7]],
            ins=[b_in.opt()], outs=[b_out.opt()])

        # ---- v dwconv groups 1,2 (PE + DVE taps), same 0.9ms slot
        with tc.tile_wait_until(0.9):
            for ct in (1, 2):
                diags_v = {t: diag_v[(t, ct)] for t in CFG["pe_taps_v"]}
                _emit_dwconv(nc, psum_dw, vpad[ct], w9v[:, ct, :],
                             diags_v, vwp, "vw", 128, CFG["pe_taps_v"], VROWS,
                             out_final=vdw[:, ct, :], tmp_pool=dwp,
                             split_halves=False)

        # ---- warm-fill: small junk matmuls (on long-dead qT) keep HAM up.
        # Block A (virtual 1.02ms, i.e. right after attn, before M) covers the
        # post-attn DVE/ACT window; block B (1.05ms, after M) covers the
        # AllGather. Fine granularity avoids overshooting past MT-ready.
        for blk, n, ms in (("a", CFG["warm_a"], 1.005), ("b", CFG["warm_b"], 1.05)):
            with tc.tile_wait_until(ms):
                for i in range(n):
                    warm_ps = psum_o.tile([128, 512], F32, tag="psout",
                                          name=f"warm{blk}{i}")
                    nc.tensor.matmul(
                        warm_ps[:, 0:C2], dgv_t[:, 0, 0, :], qT[:, i % 128, :],
                        start=True, stop=True)

        # ---- gather result + final matmul ----
        MT = pool.tile([128, 3, C], F16, tag="MT")
        bo = b_out[:].rearrange("g p c -> (g p) c")
        for kc in range(3):
            nc.sync.dma_start(MT[:, kc, :], bo[128 * kc:128 * (kc + 1), :])
        for m in range(3):
            for n in range(VPX // SUB):
                ps = psum_o.tile([128, SUB], F32, tag="psout")
                for kc in range(3):
                    nc.tensor.matmul(
                        ps[:], MT[:, kc, 128 * m:128 * (m + 1)],
                        vdw[:, kc, SUB * n:SUB * (n + 1)],
                        start=(kc == 0), stop=(kc == 2))
                ob = obuf.tile([128, SUB], F16, tag="ob")
                nc.scalar.copy(ob[:], ps[:])
                nc.sync.dma_start(out_ext.ap()[m, :, SUB * n:SUB * (n + 1)], ob[:])

    if CFG["split_waits"]:
        _split_multi_waits(nc, CFG["max_waits"])
    return nc


# ---------------- host-side sharding ----------------

def _prep_inputs(q_fea, k_fea, v_fea, wq, wk, wv, wproj, temperature, attn_w):
    q_fea = np.asarray(q_fea, np.float32)
    k_fea = np.asarray(k_fea, np.float32)
    v_fea = np.asarray(v_fea, np.float32)
    wq = np.asarray(wq, np.float32)[:, 0]      # [C,3,3]
    wk = np.asarray(wk, np.float32)[:, 0]
    wv = np.asarray(wv, np.float32)[:, 0]
    wproj = np.asarray(wproj, np.float32)[:, :, 0, 0]  # [C,C]
    temp = np.asarray(temperature, np.float32).reshape(HEADS)
    attn_w = np.asarray(attn_w, np.float32).reshape(4)

    wq9 = wq.reshape(C, 9)
    wk9 = wk.reshape(C, 9)
    wv9 = wv.reshape(C, 9)

    # dgv host layout [c, t, ct, e] (all 9 taps)
    dgv = np.zeros((128, 9, 3, 128), np.float16)
    for t in range(9):
        for ct in range(3):
            w = wv9[128 * ct:128 * (ct + 1), t].astype(np.float16)
            dgv[np.arange(128), t, ct, np.arange(128)] = w

    in_maps = []
    for core in range(8):
        b = core // 4
        g = core % 4
        ch0 = C2 * g
        r0 = VROWS * g

        def padqk(x):
            p = np.zeros((C2, 130, 130), np.float16)
            p[:, 1:129, 1:129] = x[b, ch0:ch0 + C2]
            return p

        vp = np.zeros((C, 34, 130), np.float16)
        glo = max(0, r0 - 1)
        ghi = min(H, r0 + VROWS + 1)
        vp[:, glo - (r0 - 1):ghi - (r0 - 1), 1:129] = v_fea[b, :, glo:ghi, :]

        # dgqk host layout [c, a, t, e]
        dgqk = np.zeros((C2, 2, 9, C2), np.float16)
        for t in range(9):
            dgqk[np.arange(C2), 0, t, np.arange(C2)] = \
                wq9[ch0:ch0 + C2, t].astype(np.float16)
            dgqk[np.arange(C2), 1, t, np.arange(C2)] = \
                wk9[ch0:ch0 + C2, t].astype(np.float16)

        in_maps.append({
            "qs": padqk(q_fea),
            "ks": padqk(k_fea),
            "vs": vp,
            "wq9": np.ascontiguousarray(wq9[ch0:ch0 + C2]),
            "wk9": np.ascontiguousarray(wk9[ch0:ch0 + C2]),
            "wv9": np.ascontiguousarray(wv9.reshape(3, 128, 9).transpose(1, 0, 2)),
            "dgqk": dgqk,
            "dgv": dgv,
            "wpT0": np.ascontiguousarray(wproj[:, ch0:ch0 + CH].T.astype(np.float16)),
            "wpT1": np.ascontiguousarray(wproj[:, ch0 + CH:ch0 + C2].T.astype(np.float16)),
            "tempv": np.repeat(temp[2 * g:2 * g + 2], CH)[:, None].copy(),
            "attwv": np.tile(attn_w, (C2, 1)),
            "ksv": np.tile(np.array(KS, np.float32), (C2, 1)),
        })
    return in_maps


def _assemble(results):
    out = np.zeros((B, C, H, W), np.float32)
    for core in range(8):
        b = core // 4
        r0 = VROWS * (core % 4)
        o = results[core]["out"].astype(np.float32)  # [3, 128, VPX]
        out[b, :, r0:r0 + VROWS, :] = o.reshape(C, VROWS, W)
    return out


_CACHE = {}


def kernel(**inputs) -> np.ndarray:
    if "nc" not in _CACHE:
        _CACHE["nc"] = build_kernel()
    nc = _CACHE["nc"]
    in_maps = _prep_inputs(**inputs)
    res = run_bass_kernel_spmd(nc, in_maps, core_ids=list(range(8)))
    return _assemble(res.results)


if __name__ == "__main__":
    sys.path.insert(0, "/root/problem")
    from reference import setup_inputs, reference

    inputs = setup_inputs()
    ref = np.asarray(reference(**inputs))
    got = kernel(**{k: np.asarray(v) for k, v in inputs.items()})
    rel = np.linalg.norm(got - ref) / np.linalg.norm(ref)
    print(f"Relative error: {rel:.3e}")


# revision 56
# speedup vs baseline: 1.2057x; 1.2057x over previous
"""Trainium2 Bass kernel for nn_Attention_54150947668207 (sparse channel attention).

Algorithm restructure (verified exact vs reference in fp32, rel 3.4e-7):
  - dwconv3x3 per channel on q,k,v (depthwise, SAME pad)
  - per (batch,head): attn = normalize(q) @ normalize(k)^T over pixels; the 4
    top-k masked softmaxes combine into ONE matrix A_comb = sum_i w_i*softmax_i
    (top-k via rank-count, col-scaled before ranking; exp needs no max-sub)
  - M_combT = blockdiag(A_comb)^T @ Wproj^T folds all four attn@v matmuls AND
    the 1x1 projection into ONE [384,384]@[384,px] matmul per pixel shard.

Sharding (8 cores, SPMD):
  - attention phase: core i handles batch i//4, heads {2*(i%4), 2*(i%4)+1}
  - projection phase: core i handles batch i//4, image rows 32*(i%4)..+32
  - connected by one AllGather of M_combT ([96,384] f16) in 4-core groups.

v2 schedule (from trace analysis of v1 at 280us):
  - PE stream kept dense & in one program order: qk dwconv chunks -> attn
    matmuls -> v-group0 (all-9-taps PE; covers the post-attn DVE/ACT wait) ->
    M matmuls -> v-groups 1,2 -> warm-fill -> final matmuls. Keeps HAM at
    full clock (v1 spent 102us at half clock) and fills the collective window.
  - norm reciprocals + rk broadcast DRAM bounce issued during the attn matmul
    phase (their latency fully hidden).
  - A1 head-1 block extracted straight from PSUM with a base-partition-32
    ACT copy (overwritten rows fixed by the head-0 copy) - no DRAM bounce.
  - diag-weight tables host-pre-transposed -> contiguous DMA loads.
  - input loads prefetched: first chunks on HWDGE (sync), rest on SWDGE
    (gpsimd); all dwconv transposes ride the sync HWDGE ring.
"""
import sys

for _p in ("/opt/trn_rl_repo",):
    if _p not in sys.path:
        sys.path.insert(0, _p)

import numpy as np
from contextlib import ExitStack

import concourse.bass as bass
import concourse.tile as tile
from concourse import mybir
from concourse.bass_utils import run_bass_kernel_spmd

F32 = mybir.dt.float32
F16 = mybir.dt.float16
AOT = mybir.AluOpType
ACTF = mybir.ActivationFunctionType

C = 384
HEADS = 8
CH = 48          # channels per head
H = W = 128
HW = H * W
B = 2
C2 = 96          # channels per core in attention phase (2 heads)
KS = (CH // 2, CH * 2 // 3, CH * 3 // 4, CH * 4 // 5)  # 24, 32, 36, 38

# tap order t = 3*ky + kx, offsets (dy,dx) = (ky-1, kx-1)
N_TAPS = 9
CFG = {
    "pe_taps_qk": (0, 1, 3, 4, 5, 7),   # taps done on PE (fp32 psum acc)
    "pe_taps_v": (0, 1, 3, 4, 5, 7),
    "split_waits": True,
    "max_waits": 1,
    "warm_a": 65,        # junk matmuls filling the post-attn PE window
    "warm_b": 95,        # junk matmuls bridging the collective window
    "pads_bufs": 3,
}

NCHUNK = 4            # q/k processed in 4 chunks of 32 rows
ROWS_PER_CHUNK = 32
CHUNK_PX = ROWS_PER_CHUNK * W   # 4096
SUB = 512             # psum sub-chunk width for PE dwconv
VROWS = 32            # v shard rows per core
VPX = VROWS * W       # 4096


def _split_multi_waits(nc, max_waits=1):
    """walrus in this container accepts limited sync waits per instruction;
    split extras into preceding single-wait NoOps on the same engine."""
    n = 0
    for f in nc.m.functions:
        for blk in f.blocks:
            new_insts = []
            for inst in blk.instructions:
                si = getattr(inst, "sync_info", None)
                # TENSOR_TENSOR_REDUCE's long encoding leaves no room for any
                # sync wait ("ISA wrong length" at codegen) — strip them all.
                mw = 0 if isinstance(inst, mybir.InstTensorTensorReduce) else max_waits
                if si is not None and si.on_wait and len(si.on_wait) > mw:
                    waits = list(si.on_wait)
                    if mw == 0:
                        wlist, si.on_wait = waits, []
                    else:
                        wlist, si.on_wait = waits[:-mw], waits[-mw:]
                    for wcond in wlist:
                        nop = mybir.InstNoOp(
                            name=f"I-waitsplit-{nc.next_id()}",
                            ins=[], outs=[],
                            engine=inst.engine,
                            sync_info=mybir.SyncInfo(on_wait=[wcond], on_update=[]),
                        )
                        new_insts.append(nop)
                        n += 1
                new_insts.append(inst)
            blk.instructions = new_insts
    return n


def _emit_dwconv(nc, psum_dw, xpad, w9, diags, dwp, out_tag,
                 npart, pe_taps, nrows, out_final=None, tmp_pool=None,
                 mid_emit=None, half_done=None, split_halves=True):
    """Depthwise 3x3 over nrows output rows.

    PE taps accumulate in fp32 PSUM (diag matmuls); remaining taps run as
    DVE tensor_scalar(4x) + tensor_tensor(2x) chains.
    Returns the final output AP ([npart, nrows*W] f16).
    """
    dve_taps = [t for t in range(N_TAPS) if t not in pe_taps]
    npx = nrows * W
    nsub = npx // 512
    rows_per_sub = 512 // W  # 4

    def shifted(t, r_lo, nr):
        ky, kx = divmod(t, 3)
        return xpad[:, r_lo + ky:r_lo + ky + nr, kx:kx + W]

    _ctr = [0]

    def alloc(tag, pool=None, rows=None):
        _ctr[0] += 1
        t = (pool or dwp).tile([npart, (rows or nrows) * W], F16, tag=tag,
                               name=f"{out_tag}_{tag}{_ctr[0]}")
        return t[:]

    n_dve = len(dve_taps)
    assert pe_taps
    halved_pe_out = split_halves and n_dve > 0
    if halved_pe_out:
        # two half-row tiles as PE-drain targets (halves the tag-A footprint)
        hr0 = nrows // 2
        cur_h = [alloc(out_tag + "A", rows=hr0), alloc(out_tag + "A", rows=hr0)]
        oc3_h = [c.rearrange("p (r w) -> p r w", w=W) for c in cur_h]
    else:
        cur = (out_final if (n_dve == 0 and out_final is not None)
               else alloc(out_tag + ("F" if n_dve == 0 else "A")))
        oc3 = cur.rearrange("p (r w) -> p r w", w=W)
    for s in range(nsub):
        r_lo = s * rows_per_sub
        ps = psum_dw.tile([npart, 512], F32, tag="psdw")
        for i, t in enumerate(pe_taps):
            nc.tensor.matmul(
                ps[:], diags[t], shifted(t, r_lo, 4),
                start=(i == 0), stop=(i == len(pe_taps) - 1))
        if halved_pe_out:
            half = 0 if s < nsub // 2 else 1
            rr = r_lo - half * (nrows // 2)
            nc.scalar.copy(oc3_h[half][:, rr:rr + rows_per_sub, :], ps[:])
        else:
            nc.scalar.copy(oc3[:, r_lo:r_lo + rows_per_sub, :], ps[:])
        if s == nsub // 2 - 1 and mid_emit is not None:
            mid_emit()
    if n_dve == 0:
        return cur
    if not split_halves:
        # single whole-rows chain (v groups: no squares needed, and the
        # bufs=1 v pool cannot keep the PE-out tile alive across two halves)
        oc3 = cur.rearrange("p (r w) -> p r w", w=W)
        flip = 0
        for j, t in enumerate(dve_taps):
            last = (j == n_dve - 1)
            nxt = (out_final if (last and out_final is not None)
                   else alloc(out_tag + "BA"[flip]))
            flip ^= 1
            no3 = nxt.rearrange("p (r w) -> p r w", w=W)
            tmp = alloc("dwtmp", pool=tmp_pool)
            tm3 = tmp.rearrange("p (r w) -> p r w", w=W)
            nc.vector.tensor_scalar(
                tm3, shifted(t, 0, nrows), w9[:, t:t + 1], None, AOT.mult)
            nc.vector.tensor_tensor(no3, tm3, oc3, AOT.add)
            cur, oc3 = nxt, no3
        return cur
    # DVE tap chain split into row-halves so each half's square can slot into
    # the ACT queue as soon as that half's chain is done.
    hr = nrows // 2
    for half in range(2):
        r0 = half * hr
        sub = cur_h[half]
        sc3 = sub.rearrange("p (r w) -> p r w", w=W)
        flip = 0
        for j, t in enumerate(dve_taps):
            last = (j == n_dve - 1)
            if last and out_final is not None:
                nxt = out_final[:, r0 * W:(r0 + hr) * W]
            elif last:
                if half == 0:
                    fin_full = alloc(out_tag + "F", rows=nrows)
                nxt = fin_full[:, r0 * W:(r0 + hr) * W]
            else:
                nxt = alloc(out_tag + "BA"[flip], rows=hr)
            flip ^= 1
            no3 = nxt.rearrange("p (r w) -> p r w", w=W)
            tmp = alloc("dwtmp", pool=tmp_pool, rows=hr)
            tm3 = tmp.rearrange("p (r w) -> p r w", w=W)
            nc.vector.tensor_scalar(
                tm3, shifted(t, r0, hr), w9[:, t:t + 1], None, AOT.mult)
            nc.vector.tensor_tensor(no3, tm3, sc3, AOT.add)
            sub, sc3 = nxt, no3
        if half_done is not None:
            half_done(half, (fin_full if out_final is None else out_final))
    return fin_full if out_final is None else out_final


def build_kernel():
    nc = bass.Bass("TRN2", target_bir_lowering=False, debug=False, num_devices=8)

    # ---- DRAM I/O ----
    qs = nc.declare_dram_parameter("qs", [C2, 130, 130], F16, isOutput=False)
    ks = nc.declare_dram_parameter("ks", [C2, 130, 130], F16, isOutput=False)
    vs = nc.declare_dram_parameter("vs", [C, 34, 130], F16, isOutput=False)
    wq9 = nc.declare_dram_parameter("wq9", [C2, 9], F32, isOutput=False)
    wk9 = nc.declare_dram_parameter("wk9", [C2, 9], F32, isOutput=False)
    wv9 = nc.declare_dram_parameter("wv9", [128, 3, 9], F32, isOutput=False)
    # host pre-transposed: [c, a, t, e] and [c, t, ct, e] (all 9 v taps)
    dgqk = nc.declare_dram_parameter("dgqk", [C2, 2, 9, C2], F16, isOutput=False)
    dgv = nc.declare_dram_parameter("dgv", [128, 9, 3, 128], F16, isOutput=False)
    wpT0 = nc.declare_dram_parameter("wpT0", [CH, C], F16, isOutput=False)
    wpT1 = nc.declare_dram_parameter("wpT1", [CH, C], F16, isOutput=False)
    tempv = nc.declare_dram_parameter("tempv", [C2, 1], F32, isOutput=False)
    attwv = nc.declare_dram_parameter("attwv", [C2, 4], F32, isOutput=False)
    ksv = nc.declare_dram_parameter("ksv", [C2, 4], F32, isOutput=False)
    out_ext = nc.declare_dram_parameter("out", [3, 128, VPX], F16, isOutput=True)

    with tile.TileContext(nc) as tc, ExitStack() as ctx:
        pool = ctx.enter_context(tc.tile_pool(name="sbuf", bufs=1))
        pads = ctx.enter_context(tc.tile_pool(name="pads", bufs=CFG["pads_bufs"]))
        vpads = ctx.enter_context(tc.tile_pool(name="vpads", bufs=2))
        dwp = ctx.enter_context(tc.tile_pool(name="dwp", bufs=2))
        vwp = ctx.enter_context(tc.tile_pool(name="vwp", bufs=1))
        psum_dw = ctx.enter_context(tc.tile_pool(name="psdw", bufs=4, space="PSUM"))
        psum_a = ctx.enter_context(tc.tile_pool(name="psa", bufs=1, space="PSUM"))
        psum_o = ctx.enter_context(tc.tile_pool(name="pso", bufs=2, space="PSUM"))
        obuf = ctx.enter_context(tc.tile_pool(name="obuf", bufs=3))
        dram = ctx.enter_context(tc.tile_pool(name="dram", bufs=1, space="DRAM"))

        # ---- all input loads ride the gpsimd SWDGE ring; the sync HWDGE ring
        # carries ONLY transposes + small bounces + output stores, so a
        # WAR-delayed chunk load can never block a transpose (which blocks
        # eagerly-scheduled attn matmuls on PE).
        xqk = {}
        t = pads.tile([C2, 34, 130], F16, tag="pad", name="xq0")
        nc.gpsimd.dma_start(t[:], qs.ap()[:, 0:34, :])
        xqk[("q", 0)] = t
        dgqk_t = pool.tile([C2, 2, 9, C2], F16, tag="dgqk")
        nc.gpsimd.dma_start(dgqk_t[:], dgqk.ap())
        w9q = pool.tile([C2, 9], F32); nc.gpsimd.dma_start(w9q[:], wq9.ap())
        w9k = pool.tile([C2, 9], F32); nc.gpsimd.dma_start(w9k[:], wk9.ap())
        t = pads.tile([C2, 34, 130], F16, tag="pad", name="xk0")
        nc.gpsimd.dma_start(t[:], ks.ap()[:, 0:34, :])
        xqk[("k", 0)] = t
        for nm, src in (("q", qs), ("k", ks)):
            t = pads.tile([C2, 34, 130], F16, tag="pad", name=f"x{nm}1")
            nc.gpsimd.dma_start(t[:], src.ap()[:, ROWS_PER_CHUNK:ROWS_PER_CHUNK + 34, :])
            xqk[(nm, 1)] = t

        vpad = []
        for ct in range(2):
            vp = vpads.tile([128, 34, 130], F16, tag="vpad")
            nc.gpsimd.dma_start(vp[:], vs.ap()[128 * ct:128 * (ct + 1), :, :])
            vpad.append(vp)
        dgv_t = pool.tile([128, 9, 3, 128], F16, tag="dgvt")
        nc.gpsimd.dma_start(dgv_t[:], dgv.ap())
        w9v = pool.tile([128, 3, 9], F32); nc.gpsimd.dma_start(w9v[:], wv9.ap())
        wp0 = pool.tile([CH, C], F16); nc.gpsimd.dma_start(wp0[:], wpT0.ap())
        wp1 = pool.tile([CH, C], F16); nc.gpsimd.dma_start(wp1[:], wpT1.ap())
        tmpv = pool.tile([C2, 1], F32); nc.gpsimd.dma_start(tmpv[:], tempv.ap())
        attw = pool.tile([C2, 4], F32); nc.gpsimd.dma_start(attw[:], attwv.ap())
        ks_t = pool.tile([C2, 4], F32); nc.gpsimd.dma_start(ks_t[:], ksv.ap())
        for ci in range(2, NCHUNK):
            r0 = ci * ROWS_PER_CHUNK
            for nm, src in (("q", qs), ("k", ks)):
                t = pads.tile([C2, 34, 130], F16, tag="pad", name=f"x{nm}{ci}")
                nc.gpsimd.dma_start(t[:], src.ap()[:, r0:r0 + 34, :])
                xqk[(nm, ci)] = t
        # v group 2 rides the pads rotation (reuses xq3's slot -> loads once
        # chunk-3 q dwconv is done; needed much later, at v-group-2 time)
        vp2 = pads.tile([128, 34, 130], F16, tag="pad", name="vp2")
        nc.gpsimd.dma_start(vp2[:], vs.ap()[256:384, :, :])
        vpad.append(vp2)

        diag_q = {t: dgqk_t[:, 0, t, :] for t in range(9)}
        diag_k = {t: dgqk_t[:, 1, t, :] for t in range(9)}
        diag_v = {(t, ct): dgv_t[:, t, ct, :]
                  for t in range(9) for ct in range(3)}

        # ---- q/k dwconv, 4 chunks; transposes on sync HWDGE ----
        # Squares (for the L2 norms) run on ACT per row-half. Half-0's square
        # is emitted right after its half-chain (runs while half-1 chains on
        # DVE); half-1's square is DEFERRED into the middle of the next
        # tensor's PSUM-drain stream so it never blocks those drains (the
        # v4/v5 chunk-boundary stall).
        sumsq = {"q": [], "k": []}
        vdw = pool.tile([128, 3, VPX], F16, tag="vdw")
        qT = pool.tile([128, 128, C2], F16, tag="qT")
        kT = pool.tile([128, 128, C2], F16, tag="kT")
        pending_sq = [None]
        HRW = (ROWS_PER_CHUNK // 2) * W

        def flush_pending():
            if pending_sq[0] is not None:
                emit = pending_sq[0]
                pending_sq[0] = None
                emit()

        for ci in range(NCHUNK):
            dws = {}
            for name in ("q", "k"):
                w9_ = w9q if name == "q" else w9k
                dg_ = diag_q if name == "q" else diag_k
                taps = CFG["pe_taps_qk"] if ci < NCHUNK - 1 else tuple(range(9))

                def make_sq(name, ci, half, fin):
                    sl = fin[:, half * HRW:(half + 1) * HRW]
                    ss = pool.tile([C2, 1], F32, tag=f"ss_{name}{ci}h{half}")
                    sumsq[name].append(ss)
                    sq = dwp.tile([C2, HRW], F16, tag="dwtmp",
                                  name=f"sq_{name}{ci}h{half}")

                    def emit():
                        nc.scalar.activation(sq[:], sl, ACTF.Square,
                                             accum_out=ss[:])
                    return emit

                def half_done(half, fin, name=name, ci=ci):
                    emit = make_sq(name, ci, half, fin)
                    if half == 0:
                        emit()
                    else:
                        pending_sq[0] = emit

                dw = _emit_dwconv(nc, psum_dw, xqk[(name, ci)], w9_, dg_,
                                  dwp, "dw", C2, taps, ROWS_PER_CHUNK,
                                  mid_emit=flush_pending, half_done=half_done)
                dws[name] = dw
                if len(taps) == 9:  # all-PE chunk: squares inline after drains
                    flush_pending()
                    for half in range(2):
                        make_sq(name, ci, half, dw)()
            nc.sync.dma_start_transpose(qT[:, 32 * ci:32 * ci + 32, :], dws["q"])
            nc.sync.dma_start_transpose(kT[:, 32 * ci:32 * ci + 32, :], dws["k"])
        flush_pending()

        # ---- norms: total sumsq -> rq, rk, rsc, Bc (latency hides under attn)
        nq2 = pool.tile([C2, 1], F32, tag="nq2")
        nk2 = pool.tile([C2, 1], F32, tag="nk2")
        for name, tgt in (("q", nq2), ("k", nk2)):
            parts = sumsq[name]
            nc.vector.tensor_tensor(tgt[:], parts[0][:], parts[1][:], AOT.add)
            for p in parts[2:]:
                nc.vector.tensor_tensor(tgt[:], tgt[:], p[:], AOT.add)
        rq = pool.tile([C2, 1], F32, tag="rq")
        rk = pool.tile([C2, 1], F32, tag="rk")
        for src2, dst in ((nq2, rq), (nk2, rk)):
            nc.scalar.sqrt(dst[:], src2[:])
            nc.vector.reciprocal(dst[:], dst[:])
        rk_dram = dram.tile([C2, 1], F32)
        nc.sync.dma_start(rk_dram[:], rk[:])
        Bc = pool.tile([C2, CH], F32, tag="Bc")
        rkd = rk_dram[:].rearrange("p one -> (p one)")
        nc.sync.dma_start(
            Bc[0:CH, :],
            rkd[0:CH].rearrange("(x e) -> x e", x=1).broadcast_to([CH, CH]))
        nc.sync.dma_start(
            Bc[CH:C2, :],
            rkd[CH:C2].rearrange("(x e) -> x e", x=1).broadcast_to([CH, CH]))
        rsc = pool.tile([C2, 1], F32, tag="rsc")
        nc.vector.tensor_tensor(rsc[:], rq[:], tmpv[:], AOT.mult)

        # ---- attention matmuls: virtual-time 1.0ms pushes them AFTER the
        # whole qk dwconv phase in the PE queue — otherwise the scheduler
        # parks them at chunk boundaries where they stall on the transpose
        # that trails each chunk's DVE tap chain.
        ps_attn = psum_a.tile([C2, C2], F32, tag="psattn")
        with tc.tile_wait_until(1.0):
            for j in range(128):
                nc.tensor.matmul(ps_attn[:], qT[:, j, :], kT[:, j, :],
                                 start=(j == 0), stop=(j == 127))

        # ---- post-attention (small, DVE/ACT) ----
        # A1[r, d] = attn[r, head(r)*48 + d]; head1 block via base-partition-32
        # psum copy (rows 32:48 garbage, then overwritten by the head0 copy)
        A1 = pool.tile([C2, CH], F32, tag="A1")
        nc.scalar.copy(A1[32:64, :], ps_attn[32:64, CH:C2])
        nc.scalar.copy(A1[64:C2, :], ps_attn[64:C2, CH:C2])
        nc.scalar.copy(A1[0:CH, :], ps_attn[0:CH, 0:CH])
        nc.vector.tensor_tensor(A1[:], A1[:], Bc[:], AOT.mult)
        # E before G: ACT exponentiates while DVE does the rank count
        E = pool.tile([C2, CH], F32, tag="E")
        nc.scalar.activation(E[:], A1[:], ACTF.Exp, scale=rsc[:])
        # rank count: G[r, d, e] = A1[r, e] > A1[r, d]  (free dims d,e)
        G = pool.tile([C2, CH, CH], F16, tag="G")
        in_e = A1[:].rearrange("p (x e) -> p x e", x=1).broadcast_to([C2, CH, CH])
        in_d = A1[:].rearrange("p (d x) -> p d x", x=1).broadcast_to([C2, CH, CH])
        nc.vector.tensor_tensor(G[:], in_e, in_d, AOT.is_gt)
        cnt = pool.tile([C2, CH], F32, tag="cnt")
        nc.vector.tensor_reduce(cnt[:], G[:], axis=mybir.AxisListType.X, op=AOT.add)
        # 4 masked softmaxes, batched over the k axis ([C2, 4, CH] tiles)
        M4 = pool.tile([C2, 4, CH], F16, tag="M4")
        cnt_b = cnt[:].rearrange("p (x e) -> p x e", x=1).broadcast_to([C2, 4, CH])
        ks_b = ks_t[:].rearrange("p (d x) -> p d x", x=1).broadcast_to([C2, 4, CH])
        nc.vector.tensor_tensor(M4[:], cnt_b, ks_b, AOT.is_lt)
        N4 = pool.tile([C2, 4, CH], F32, tag="N4")
        E_b = E[:].rearrange("p (x e) -> p x e", x=1).broadcast_to([C2, 4, CH])
        nc.vector.tensor_tensor(N4[:], E_b, M4[:], AOT.mult)
        den4 = pool.tile([C2, 4], F32, tag="den4")
        nc.vector.tensor_reduce(den4[:], N4[:], axis=mybir.AxisListType.X,
                                op=AOT.add)
        rw4 = pool.tile([C2, 4], F32, tag="rw4")
        nc.vector.reciprocal(rw4[:], den4[:])
        nc.vector.tensor_tensor(rw4[:], rw4[:], attw[:], AOT.mult)
        P4 = pool.tile([C2, 4, CH], F32, tag="P4")
        rw_b = rw4[:].rearrange("p (d x) -> p d x", x=1).broadcast_to([C2, 4, CH])
        nc.vector.tensor_tensor(P4[:], N4[:], rw_b, AOT.mult)
        t01 = pool.tile([C2, CH], F32, tag="t01")
        nc.vector.tensor_tensor(t01[:], P4[:, 0, :], P4[:, 1, :], AOT.add)
        t23 = pool.tile([C2, CH], F32, tag="t23")
        nc.vector.tensor_tensor(t23[:], P4[:, 2, :], P4[:, 3, :], AOT.add)
        Acc = pool.tile([C2, CH], F16, tag="Acc")
        nc.vector.tensor_tensor(Acc[:], t01[:], t23[:], AOT.add)
        # head1 rows to base partition 0 (head0 is Acc[0:CH] in place)
        Ah1 = pool.tile([CH, CH], F16, tag="Ah1")
        nc.gpsimd.dma_start(Ah1[:], Acc[CH:C2, :])

        # ---- v dwconv: virtual 0.9ms = right after the qk phase, before the
        # attn block — the scheduler uses it to cover the last chunk's
        # DVE-chain + transpose tail.
        with tc.tile_wait_until(0.9):
            _emit_dwconv(nc, psum_dw, vpad[0], w9v[:, 0, :],
                         {t: diag_v[(t, 0)] for t in range(9)}, vwp, "vw",
                         128, tuple(range(9)), VROWS, out_final=vdw[:, 0, :])

        # ---- M_combT partials -> DRAM -> AllGather within 4-core groups
        b_in = dram.tile([C2, C], F16)
        b_out = dram.tile([4, C2, C], F16)
        for h, (ah, wp) in enumerate(((Acc[0:CH, :], wp0), (Ah1[:], wp1))):
            ps = psum_a.tile([CH, C], F32, tag="psmc")
            nc.tensor.matmul(ps[:], ah, wp[:], start=True, stop=True)
            mt_h = pool.tile([CH, C], F16, tag=f"mth{h}")
            nc.scalar.copy(mt_h[:], ps[:])
            nc.sync.dma_start(b_in[CH * h:CH * (h + 1), :], mt_h[:])
        nc.gpsimd.collective_compute(
            "AllGather", AOT.bypass,
            replica_groups=[[0, 1, 2, 3], [4, 5, 6, 7]],
            ins=[b_in.opt()], outs=[b_out.opt()])

        # ---- v dwconv groups 1,2, same 0.9ms slot. Group 2 runs all-PE: a
        # 6+3 group-2 would WAR-stall its PE-out on group-1's DVE chain
        # through the bufs=1 v pool (~7us PE gap).
        with tc.tile_wait_until(0.9):
            diags_v = {t: diag_v[(t, 1)] for t in CFG["pe_taps_v"]}
            _emit_dwconv(nc, psum_dw, vpad[1], w9v[:, 1, :],
                         diags_v, vwp, "vw", 128, CFG["pe_taps_v"], VROWS,
                         out_final=vdw[:, 1, :], tmp_pool=dwp,
                         split_halves=False)
            _emit_dwconv(nc, psum_dw, vpad[2], w9v[:, 2, :],
                         {t: diag_v[(t, 2)] for t in range(9)}, vwp, "vw",
                         128, tuple(range(9)), VROWS, out_final=vdw[:, 2, :])

        # ---- warm-fill: small junk matmuls (on long-dead qT) keep HAM up.
        # Block A (virtual 1.02ms, i.e. right after attn, before M) covers the
        # post-attn DVE/ACT window; block B (1.05ms, after M) covers the
        # AllGather. Fine granularity avoids overshooting past MT-ready.
        for blk, n, ms in (("a", CFG["warm_a"], 1.005), ("b", CFG["warm_b"], 1.05)):
            with tc.tile_wait_until(ms):
                for i in range(n):
                    warm_ps = psum_o.tile([128, 512], F32, tag="psout",
                                          name=f"warm{blk}{i}")
                    nc.tensor.matmul(
                        warm_ps[:, 0:C2], dgv_t[:, 0, 0, :], qT[:, i % 128, :],
                        start=True, stop=True)

        # ---- gather result + final matmul ----
        MT = pool.tile([128, 3, C], F16, tag="MT")
        bo = b_out[:].rearrange("g p c -> (g p) c")
        for kc in range(3):
            nc.sync.dma_start(MT[:, kc, :], bo[128 * kc:128 * (kc + 1), :])
        for m in range(3):
            for n in range(VPX // SUB):
                ps = psum_o.tile([128, SUB], F32, tag="psout")
                for kc in range(3):
                    nc.tensor.matmul(
                        ps[:], MT[:, kc, 128 * m:128 * (m + 1)],
                        vdw[:, kc, SUB * n:SUB * (n + 1)],
                        start=(kc == 0), stop=(kc == 2))
                ob = obuf.tile([128, SUB], F16, tag="ob")
                nc.scalar.copy(ob[:], ps[:])
                nc.sync.dma_start(out_ext.ap()[m, :, SUB * n:SUB * (n + 1)], ob[:])

    if CFG["split_waits"]:
        _split_multi_waits(nc, CFG["max_waits"])
    return nc


# ---------------- host-side sharding ----------------

def _prep_inputs(q_fea, k_fea, v_fea, wq, wk, wv, wproj, temperature, attn_w):
    q_fea = np.asarray(q_fea, np.float32)
    k_fea = np.asarray(k_fea, np.float32)
    v_fea = np.asarray(v_fea, np.float32)
    wq = np.asarray(wq, np.float32)[:, 0]      # [C,3,3]
    wk = np.asarray(wk, np.float32)[:, 0]
    wv = np.asarray(wv, np.float32)[:, 0]
    wproj = np.asarray(wproj, np.float32)[:, :, 0, 0]  # [C,C]
    temp = np.asarray(temperature, np.float32).reshape(HEADS)
    attn_w = np.asarray(attn_w, np.float32).reshape(4)

    wq9 = wq.reshape(C, 9)
    wk9 = wk.reshape(C, 9)
    wv9 = wv.reshape(C, 9)

    # dgv host layout [c, t, ct, e] (all 9 taps)
    dgv = np.zeros((128, 9, 3, 128), np.float16)
    for t in range(9):
        for ct in range(3):
            w = wv9[128 * ct:128 * (ct + 1), t].astype(np.float16)
            dgv[np.arange(128), t, ct, np.arange(128)] = w

    in_maps = []
    for core in range(8):
        b = core // 4
        g = core % 4
        ch0 = C2 * g
        r0 = VROWS * g

        def padqk(x):
            p = np.zeros((C2, 130, 130), np.float16)
            p[:, 1:129, 1:129] = x[b, ch0:ch0 + C2]
            return p

        vp = np.zeros((C, 34, 130), np.float16)
        glo = max(0, r0 - 1)
        ghi = min(H, r0 + VROWS + 1)
        vp[:, glo - (r0 - 1):ghi - (r0 - 1), 1:129] = v_fea[b, :, glo:ghi, :]

        # dgqk host layout [c, a, t, e]
        dgqk = np.zeros((C2, 2, 9, C2), np.float16)
        for t in range(9):
            dgqk[np.arange(C2), 0, t, np.arange(C2)] = \
                wq9[ch0:ch0 + C2, t].astype(np.float16)
            dgqk[np.arange(C2), 1, t, np.arange(C2)] = \
                wk9[ch0:ch0 + C2, t].astype(np.float16)

        in_maps.append({
            "qs": padqk(q_fea),
            "ks": padqk(k_fea),
            "vs": vp,
            "wq9": np.ascontiguousarray(wq9[ch0:ch0 + C2]),
            "wk9": np.ascontiguousarray(wk9[ch0:ch0 + C2]),
            "wv9": np.ascontiguousarray(wv9.reshape(3, 128, 9).transpose(1, 0, 2)),
            "dgqk": dgqk,
            "dgv": dgv,
            "wpT0": np.ascontiguousarray(wproj[:, ch0:ch0 + CH].T.astype(np.float16)),
            "wpT1": np.ascontiguousarray(wproj[:, ch0 + CH:ch0 + C2].T.astype(np.float16)),
            "tempv": np.repeat(temp[2 * g:2 * g + 2], CH)[:, None].copy(),
            "attwv": np.tile(attn_w, (C2, 1)),
            "ksv": np.tile(np.array(KS, np.float32), (C2, 1)),
        })
    return in_maps


def _assemble(results):
    out = np.zeros((B, C, H, W), np.float32)
    for core in range(8):
        b = core // 4
        r0 = VROWS * (core % 4)
        o = results[core]["out"].astype(np.float32)  # [3, 128, VPX]
        out[b, :, r0:r0 + VROWS, :] = o.reshape(C, VROWS, W)
    return out


_CACHE = {}


def kernel(**inputs) -> np.ndarray:
    if "nc" not in _CACHE:
        _CACHE["nc"] = build_kernel()
    nc = _CACHE["nc"]
    in_maps = _prep_inputs(**inputs)
    res = run_bass_kernel_spmd(nc, in_maps, core_ids=list(range(8)))
    return _assemble(res.results)


if __name__ == "__main__":
    sys.path.insert(0, "/root/problem")
    from reference import setup_inputs, reference

    inputs = setup_inputs()
    ref = np.asarray(reference(**inputs))
    got = kernel(**{k: np.asarray(v) for k, v in inputs.items()})
    rel = np.linalg.norm(got - ref) / np.linalg.norm(ref)
    print(f"Relative error: {rel:.3e}")
